# revision 18
# baseline (speedup 1.0000x reference)
"""Diagonal-masked multi-head self-attention on 8 TRN2 NeuronCores.

Sharding: core c handles batch b = c // 2 and heads h0 = (c % 2) * 8 .. +8
(data parallel on B=4, tensor parallel over the 16 heads).  Each core
computes a partial output [S, D]; the host sums the two half-head partials
per batch and adds the output bias.

Per-core dataflow (bf16 matmuls, fp32 PSUM accumulation):
  - Host pre-transposes activations/weights so every matmul operand is
    already in its natural [K-on-partitions, free] layout.
  - Q/K projections produce QH^T / KH^T [dk, seq]; V produces VH [seq, dk].
    KH^T is stored twice with the other head's rows zeroed so score
    matmuls run with full K=128 weights (enables fast weight load).
  - Scores are computed transposed (S^T[t, q]); exp runs on the scalar
    engine straight out of PSUM (scale=1/sqrt(dk) folded in); the
    diagonal mask multiplies the one diagonal 128x128 block by (1 - I).
  - P^T @ V is computed as O^T with a ones column folded into the V
    weights, so each head's softmax denominator falls out of the same
    matmul (row 64 of each half's PSUM tile).
  - Normalization broadcasts the reciprocal denominator across
    partitions on GpSimd and multiplies during the PSUM->SBUF copy.
  - The output projection contracts O^T directly (it is already the
    lhsT the PE wants).
"""

import numpy as np
import ml_dtypes

B, S, D, H = 4, 2048, 1024, 16
DK = D // H
N_CORES = 8
HEADS_PER_CORE = H // 2


def build_attention_core(S=2048, DIN=1024, NH=8, DOUT=1024, aug_bias=False):
    """Build the per-core Tile program (pair-pipelined).

    Emission order: V projection, Q/K projection for pair 0, then for each
    pair its attention stream followed by the next pair's Q/K projection
    (which the scheduler pulls into the attention phase's PE gaps), and
    finally the output projection.
    """
    import concourse.bacc as bacc
    import concourse.bass as bass
    import concourse.mybir as mybir
    import concourse.tile as tile

    fp32 = mybir.dt.float32
    bf16 = mybir.dt.bfloat16

    NP = NH // 2              # head pairs
    DC = NH * DK              # concat head dim on this core
    VW = 128                  # per-head V slot: [V(64) ones(1) pad(63)=1]
    NT = S // 128             # t tiles (key/value positions)
    NQ = S // 512             # q chunks of 512
    KA = DIN + 1 if aug_bias else DIN
    NK = (KA + 127) // 128    # contraction tiles for projections
    ND = (DOUT + 511) // 512  # output-dim chunks
    DCH = min(512, DOUT)

    assert S % 512 == 0 and DIN % 128 == 0 and DOUT % 512 in (0, DOUT)

    nc = bacc.Bacc(None, target_bir_lowering=False, debug=False)

    xq = nc.dram_tensor("xq", [KA, S], bf16, kind="ExternalInput")
    xk = nc.dram_tensor("xk", [KA, S], bf16, kind="ExternalInput")
    xv = nc.dram_tensor("xv", [KA, S], bf16, kind="ExternalInput")
    wq = nc.dram_tensor("wq", [KA, DC], bf16, kind="ExternalInput")
    wk = nc.dram_tensor("wk", [KA, DC], bf16, kind="ExternalInput")
    wv = nc.dram_tensor("wv", [KA, DC], bf16, kind="ExternalInput")
    wo = nc.dram_tensor("wo", [DC, DOUT], bf16, kind="ExternalInput")
    eyec = nc.dram_tensor("eyec", [128, 128], bf16, kind="ExternalInput")
    outp = nc.dram_tensor("outp", [S, DOUT], fp32, kind="ExternalOutput")

    def ksz(k):  # rows in contraction tile k
        return min(128, KA - k * 128)

    with tile.TileContext(nc) as tc:
        with (
            tc.tile_pool(name="persist", bufs=1) as persist,
            tc.tile_pool(name="xin", bufs=NK + 2) as xin,
            tc.tile_pool(name="win", bufs=1) as win,
            tc.tile_pool(name="epool", bufs=6) as epool,
            tc.tile_pool(name="npool", bufs=2) as npool,
            tc.tile_pool(name="opool", bufs=2) as opool,
            tc.tile_pool(name="scps", bufs=4, space="PSUM") as scps,
            tc.tile_pool(name="otaps", bufs=2, space="PSUM") as otaps,
            tc.tile_pool(name="otbps", bufs=2, space="PSUM") as otbps,
        ):
            # ---- persistent SBUF tensors (per-pair for dep isolation) ---
            qht, khtp, ot = [], [], []
            for p in range(NP):
                qht_p = persist.tile([128, S], bf16, tag=f"qht{p}")
                khtp_p = persist.tile([128, 2 * S], bf16, tag=f"khtp{p}")
                ot_p = persist.tile([128, S], bf16, tag=f"ot{p}")
                qht.append(qht_p)
                khtp.append(khtp_p)
                ot.append(ot_p)
            vh = persist.tile([128, NT * NH * VW], bf16, tag="vh")
            eye = persist.tile([128, 128], bf16, tag="eye")
            wo_sb = persist.tile([128, NP * DOUT], bf16, tag="wo")   # pair-major

            nc.sync.dma_start(eye[:], eyec[:])
            nc.vector.memset(vh[:], 1.0)
            for p in range(NP):
                nc.vector.memset(khtp[p][:], 0.0)
                nc.sync.dma_start(
                    wo_sb[:, p * DOUT:(p + 1) * DOUT],
                    wo[p * 128:(p + 1) * 128, :],
                )

            # weight tiles (resident, small)
            wts = {}
            for which, wdram in (("q", wq), ("k", wk), ("v", wv)):
                wts[which] = []
                for k in range(NK):
                    wtile = win.tile([128, DC], bf16, tag=f"w{which}{k}")
                    nc.sync.dma_start(
                        wtile[: ksz(k), :], wdram[k * 128: k * 128 + ksz(k), :]
                    )
                    wts[which].append(wtile)

            # ---- V projection (whole, first) --------------------------
            xt = []
            for k in range(NK):
                xtile = xin.tile([128, S], bf16, tag="xt")
                nc.sync.dma_start(xtile[: ksz(k), :], xv[k * 128: k * 128 + ksz(k), :])
                xt.append(xtile)
            for t in range(NT):
                ps = scps.tile([128, DC], fp32, tag="sc")
                for k in range(NK):
                    nc.tensor.matmul(
                        ps[:],
                        xt[k][: ksz(k), t * 128:(t + 1) * 128],
                        wts["v"][k][: ksz(k), :],
                        start=(k == 0),
                        stop=(k == NK - 1),
                    )
                base = t * NH * VW
                nc.vector.tensor_copy(
                    vh[:, base: base + NH * VW].rearrange(
                        "p (h c) -> p h c", c=VW
                    )[:, :, 0:DK],
                    ps[:].rearrange("p (h c) -> p h c", c=DK),
                )

            def qk_proj(p):
                """Project Q and K for pair p (streams x tiles)."""
                for which, xdram in (("q", xq), ("k", xk)):
                    xt = []
                    for k in range(NK):
                        xtile = xin.tile([128, S], bf16, tag="xt")
                        nc.sync.dma_start(
                            xtile[: ksz(k), :], xdram[k * 128: k * 128 + ksz(k), :]
                        )
                        xt.append(xtile)
                    for n in range(NQ):
                        ps = scps.tile([128, 512], fp32, tag="sc")
                        for k in range(NK):
                            nc.tensor.matmul(
                                ps[:],
                                wts[which][k][: ksz(k), p * 128:(p + 1) * 128],
                                xt[k][: ksz(k), n * 512:(n + 1) * 512],
                                start=(k == 0),
                                stop=(k == NK - 1),
                            )
                        if which == "q":
                            nc.vector.tensor_copy(
                                qht[p][:, n * 512:(n + 1) * 512], ps[:]
                            )
                        else:
                            nc.vector.tensor_copy(
                                khtp[p][0:64, n * 512: n * 512 + 512], ps[0:64, :]
                            )
                            nc.vector.tensor_copy(
                                khtp[p][64:128, S + n * 512: S + n * 512 + 512],
                                ps[64:128, :],
                            )

            qk_proj(0)

            # ---- attention, pair-pipelined ----------------------------
            scale = float(1.0 / np.sqrt(DK))
            for p in range(NP):
                for n in range(NQ):
                    qof = n * 512
                    ota = otaps.tile([128, 512], fp32, tag="ota")
                    otb = otbps.tile([128, 512], fp32, tag="otb")
                    for t in range(NT):
                        sa = scps.tile([128, 512], fp32, tag="sc")
                        sb = scps.tile([128, 512], fp32, tag="sc")
                        nc.tensor.matmul(
                            sa[:],
                            khtp[p][:, t * 128:(t + 1) * 128],
                            qht[p][:, qof: qof + 512],
                            start=True, stop=True,
                        )
                        nc.tensor.matmul(
                            sb[:],
                            khtp[p][:, S + t * 128: S + (t + 1) * 128],
                            qht[p][:, qof: qof + 512],
                            start=True, stop=True,
                        )
                        ea = epool.tile([128, 512], bf16, tag="e")
                        eb = epool.tile([128, 512], bf16, tag="e")
                        nc.scalar.activation(
                            ea[:], sa[:], mybir.ActivationFunctionType.Exp,
                            scale=scale,
                        )
                        nc.scalar.activation(
                            eb[:], sb[:], mybir.ActivationFunctionType.Exp,
                            scale=scale,
                        )
                        off = t * 128 - n * 512
                        if 0 <= off < 512:
                            nc.vector.tensor_mul(
                                ea[:, off: off + 128], ea[:, off: off + 128], eye[:]
                            )
                            nc.vector.tensor_mul(
                                eb[:, off: off + 128], eb[:, off: off + 128], eye[:]
                            )
                        vbase = t * NH * VW
                        nc.tensor.matmul(
                            ota[:],
                            vh[:, vbase + (2 * p) * VW: vbase + (2 * p + 1) * VW],
                            ea[:],
                            start=(t == 0), stop=(t == NT - 1),
                        )
                        nc.tensor.matmul(
                            otb[:],
                            vh[:, vbase + (2 * p + 1) * VW: vbase + (2 * p + 2) * VW],
                            eb[:],
                            start=(t == 0), stop=(t == NT - 1),
                        )
                    # normalize; HW partition_broadcast reads physical
                    # partition 0, so bounce the recip rows down via DMA.
                    rd = npool.tile([128, 1024], fp32, tag="rd")
                    nc.vector.reciprocal(rd[64:65, 0:512], ota[64:65, :])
                    nc.vector.reciprocal(rd[64:65, 512:1024], otb[64:65, :])
                    nc.sync.dma_start(rd[0:1, 0:512], rd[64:65, 0:512])
                    nc.sync.dma_start(rd[0:1, 512:1024], rd[64:65, 512:1024])
                    bca = npool.tile([64, 512], fp32, tag="bca")
                    bcb = npool.tile([64, 512], fp32, tag="bcb")
                    nc.gpsimd.partition_broadcast(bca[:], rd[0:1, 0:512], channels=64)
                    nc.gpsimd.partition_broadcast(bcb[:], rd[0:1, 512:1024], channels=64)
                    nc.vector.tensor_mul(
                        ot[p][0:64, qof: qof + 512], ota[0:64, :], bca[:]
                    )
                    tmpb = npool.tile([64, 512], bf16, tag="tmpb")
                    nc.vector.tensor_mul(tmpb[:], otb[0:64, :], bcb[:])
                    nc.sync.dma_start(ot[p][64:128, qof: qof + 512], tmpb[:])
                    if n == 0 and p + 1 < NP:
                        qk_proj(p + 1)  # mid-pair: DMAs prefetch, MMs fill gaps
                    if p == NP - 1:
                        # output projection for this q chunk (all pairs done)
                        for qs in range(4):
                            qt = n * 4 + qs
                            osb = opool.tile([128, DOUT], fp32, tag="osb")
                            for nd in range(ND):
                                ps = scps.tile([128, DCH], fp32, tag="sc")
                                for pp in range(NP):
                                    nc.tensor.matmul(
                                        ps[:],
                                        ot[pp][:, qt * 128:(qt + 1) * 128],
                                        wo_sb[:, pp * DOUT + nd * DCH: pp * DOUT + nd * DCH + DCH],
                                        start=(pp == 0), stop=(pp == NP - 1),
                                    )
                                nc.vector.tensor_copy(
                                    osb[:, nd * DCH:(nd + 1) * DCH], ps[:]
                                )
                            nc.sync.dma_start(
                                outp[qt * 128:(qt + 1) * 128, :], osb[:]
                            )

    nc.compile()
    return nc


def _bf16(a):
    return np.ascontiguousarray(a).astype(ml_dtypes.bfloat16)


def _prep_core_inputs(q, k, v, Wq, bq, Wk, bk, Wv, bv, Wo, aug_bias):
    """Per-core host-side slicing/transposition. Returns list of 8 dicts."""
    eyec = _bf16(1.0 - np.eye(128, dtype=np.float32))
    maps = []
    for c in range(N_CORES):
        b = c // 2
        h0 = (c % 2) * HEADS_PER_CORE
        r0, r1 = h0 * DK, (h0 + HEADS_PER_CORE) * DK
        m = {}
        for name, x in (("xq", q[b]), ("xk", k[b]), ("xv", v[b])):
            xt = x.T  # [D, S]
            if aug_bias:
                xt = np.concatenate([xt, np.ones((1, S), np.float32)], axis=0)
            m[name] = _bf16(xt)
        for name, W, bias in (("wq", Wq, bq), ("wk", Wk, bk), ("wv", Wv, bv)):
            wt = W[r0:r1, :].T  # [D, DC]
            if aug_bias:
                wt = np.concatenate([wt, bias[None, r0:r1]], axis=0)
            m[name] = _bf16(wt)
        m["wo"] = _bf16(Wo[:, r0:r1].T)  # [DC, D]
        m["eyec"] = eyec
        maps.append(m)
    return maps


_PROGRAM_CACHE = {}


def _get_program(aug_bias):
    if aug_bias not in _PROGRAM_CACHE:
        _PROGRAM_CACHE[aug_bias] = build_attention_core(
            S=S, DIN=D, NH=HEADS_PER_CORE, DOUT=D, aug_bias=aug_bias
        )
    return _PROGRAM_CACHE[aug_bias]


def _reference_fallback(q, k, v, Wq, bq, Wk, bk, Wv, bv, Wo, bo, mask):
    """Pure-numpy fallback for unexpected mask patterns."""
    out = np.empty((B, S, D), np.float32)
    msk = np.broadcast_to(mask.reshape(mask.shape[-2], mask.shape[-1]), (S, S))
    for b in range(B):
        qh = (q[b] @ Wq.T + bq).reshape(S, H, DK).transpose(1, 0, 2)
        kh = (k[b] @ Wk.T + bk).reshape(S, H, DK).transpose(1, 0, 2)
        vh = (v[b] @ Wv.T + bv).reshape(S, H, DK).transpose(1, 0, 2)
        acc = np.empty((H, S, DK), np.float32)
        for h in range(H):
            s = (qh[h] @ kh[h].T) / np.float32(np.sqrt(DK))
            s = np.where(msk == 0, np.finfo(np.float32).min, s)
            s = s - s.max(axis=-1, keepdims=True)
            e = np.exp(s)
            p = e / e.sum(axis=-1, keepdims=True)
            acc[h] = p @ vh[h]
        o = acc.transpose(1, 0, 2).reshape(S, D)
        out[b] = o @ Wo.T + bo
    return out


def kernel(q, k, v, Wq, bq, Wk, bk, Wv, bv, Wo, bo, mask, _trace=False):
    from concourse.bass_utils import run_bass_kernel_spmd

    q = np.asarray(q, np.float32)
    k = np.asarray(k, np.float32)
    v = np.asarray(v, np.float32)
    Wq, bq = np.asarray(Wq, np.float32), np.asarray(bq, np.float32)
    Wk, bk = np.asarray(Wk, np.float32), np.asarray(bk, np.float32)
    Wv, bv = np.asarray(Wv, np.float32), np.asarray(bv, np.float32)
    Wo, bo = np.asarray(Wo, np.float32), np.asarray(bo, np.float32)
    mask = np.asarray(mask)

    expected_mask = 1 - np.eye(S, dtype=np.int32)
    if not np.array_equal(mask.reshape(-1, S, S)[0].astype(np.int32), expected_mask):
        return _reference_fallback(q, k, v, Wq, bq, Wk, bk, Wv, bv, Wo, bo, mask)

    aug_bias = bool(np.any(bq) or np.any(bk) or np.any(bv))
    nc = _get_program(aug_bias)
    in_maps = _prep_core_inputs(q, k, v, Wq, bq, Wk, bk, Wv, bv, Wo, aug_bias)
    res = run_bass_kernel_spmd(
        nc, in_maps, core_ids=list(range(N_CORES)), trace=_trace
    )
    out = np.empty((B, S, D), np.float32)
    for b in range(B):
        out[b] = res.results[2 * b]["outp"] + res.results[2 * b + 1]["outp"] + bo
    if _trace:
        kernel.last_results = res
    return out


# revision 20
# speedup vs baseline: 1.0515x; 1.0515x over previous
"""Diagonal-masked multi-head self-attention on 8 TRN2 NeuronCores.

Sharding: core c handles batch b = c // 2 and heads h0 = (c % 2) * 8 .. +8
(data parallel on B=4, tensor parallel over the 16 heads).  Each core
computes a partial output [S, D]; the host sums the two half-head partials
per batch and adds the output bias.

Per-core dataflow (bf16 matmuls, fp32 PSUM accumulation):
  - Host pre-transposes activations/weights so every matmul operand is
    already in its natural [K-on-partitions, free] layout.
  - Q/K projections produce QH^T / KH^T [dk, seq]; V produces VH [seq, dk].
    KH^T is stored twice with the other head's rows zeroed so score
    matmuls run with full K=128 weights (enables fast weight load).
  - Scores are computed transposed (S^T[t, q]); exp runs on the scalar
    engine straight out of PSUM (scale=1/sqrt(dk) folded in); the
    diagonal mask multiplies the one diagonal 128x128 block by (1 - I).
  - P^T @ V is computed as O^T with a ones column folded into the V
    weights, so each head's softmax denominator falls out of the same
    matmul (row 64 of each half's PSUM tile).
  - Normalization broadcasts the reciprocal denominator across
    partitions on GpSimd and multiplies during the PSUM->SBUF copy.
  - The output projection contracts O^T directly (it is already the
    lhsT the PE wants).
"""

import numpy as np
import ml_dtypes

B, S, D, H = 4, 2048, 1024, 16
DK = D // H
N_CORES = 8
HEADS_PER_CORE = H // 2


def build_attention_core(S=2048, DIN=1024, NH=8, DOUT=1024, aug_bias=False):
    """Build the per-core Tile program (pair-pipelined).

    Emission order: V projection, Q/K projection for pair 0, then for each
    pair its attention stream followed by the next pair's Q/K projection
    (which the scheduler pulls into the attention phase's PE gaps), and
    finally the output projection.
    """
    import concourse.bacc as bacc
    import concourse.bass as bass
    import concourse.mybir as mybir
    import concourse.tile as tile

    fp32 = mybir.dt.float32
    bf16 = mybir.dt.bfloat16

    NP = NH // 2              # head pairs
    DC = NH * DK              # concat head dim on this core
    VW = 128                  # per-head V slot: [V(64) ones(1) pad(63)=1]
    NT = S // 128             # t tiles (key/value positions)
    NQ = S // 512             # q chunks of 512
    KA = DIN + 1 if aug_bias else DIN
    NK = (KA + 127) // 128    # contraction tiles for projections
    ND = (DOUT + 511) // 512  # output-dim chunks
    DCH = min(512, DOUT)

    assert S % 512 == 0 and DIN % 128 == 0 and DOUT % 512 in (0, DOUT)

    nc = bacc.Bacc(None, target_bir_lowering=False, debug=False)

    xq = nc.dram_tensor("xq", [KA, S], bf16, kind="ExternalInput")
    xk = nc.dram_tensor("xk", [KA, S], bf16, kind="ExternalInput")
    xv = nc.dram_tensor("xv", [KA, S], bf16, kind="ExternalInput")
    wq = nc.dram_tensor("wq", [KA, DC], bf16, kind="ExternalInput")
    wk = nc.dram_tensor("wk", [KA, DC], bf16, kind="ExternalInput")
    wv = nc.dram_tensor("wv", [KA, DC], bf16, kind="ExternalInput")
    wo = nc.dram_tensor("wo", [DC, DOUT], bf16, kind="ExternalInput")
    eyec = nc.dram_tensor("eyec", [128, 128], bf16, kind="ExternalInput")
    outp = nc.dram_tensor("outp", [S, DOUT], fp32, kind="ExternalOutput")

    def ksz(k):  # rows in contraction tile k
        return min(128, KA - k * 128)

    with tile.TileContext(nc) as tc:
        with (
            tc.tile_pool(name="persist", bufs=1) as persist,
            tc.tile_pool(name="xin", bufs=NK + 2) as xin,
            tc.tile_pool(name="win", bufs=1) as win,
            tc.tile_pool(name="epool", bufs=6) as epool,
            tc.tile_pool(name="npool", bufs=2) as npool,
            tc.tile_pool(name="opool", bufs=2) as opool,
            tc.tile_pool(name="scps", bufs=4, space="PSUM") as scps,
            tc.tile_pool(name="otaps", bufs=2, space="PSUM") as otaps,
            tc.tile_pool(name="otbps", bufs=2, space="PSUM") as otbps,
        ):
            # ---- persistent SBUF tensors (per-pair for dep isolation) ---
            qht, khtp, ot = [], [], []
            for p in range(NP):
                qht_p = persist.tile([128, S], bf16, tag=f"qht{p}")
                khtp_p = persist.tile([128, 2 * S], bf16, tag=f"khtp{p}")
                ot_p = persist.tile([128, S], bf16, tag=f"ot{p}")
                qht.append(qht_p)
                khtp.append(khtp_p)
                ot.append(ot_p)
            vh = persist.tile([128, NT * NH * VW], bf16, tag="vh")
            eye = persist.tile([128, 128], bf16, tag="eye")
            wo_sb = persist.tile([128, NP * DOUT], bf16, tag="wo")   # pair-major

            nc.sync.dma_start(eye[:], eyec[:])
            nc.vector.memset(vh[:], 1.0)
            for p in range(NP):
                nc.vector.memset(khtp[p][:], 0.0)
                nc.sync.dma_start(
                    wo_sb[:, p * DOUT:(p + 1) * DOUT],
                    wo[p * 128:(p + 1) * 128, :],
                )

            # weight tiles (resident, small)
            wts = {}
            for which, wdram in (("q", wq), ("k", wk), ("v", wv)):
                wts[which] = []
                for k in range(NK):
                    wtile = win.tile([128, DC], bf16, tag=f"w{which}{k}")
                    nc.sync.dma_start(
                        wtile[: ksz(k), :], wdram[k * 128: k * 128 + ksz(k), :]
                    )
                    wts[which].append(wtile)

            # ---- V projection (whole, first) --------------------------
            xt = []
            for k in range(NK):
                xtile = xin.tile([128, S], bf16, tag="xt")
                nc.sync.dma_start(xtile[: ksz(k), :], xv[k * 128: k * 128 + ksz(k), :])
                xt.append(xtile)
            for t in range(NT):
                ps = scps.tile([128, DC], fp32, tag="sc")
                for k in range(NK):
                    nc.tensor.matmul(
                        ps[:],
                        xt[k][: ksz(k), t * 128:(t + 1) * 128],
                        wts["v"][k][: ksz(k), :],
                        start=(k == 0),
                        stop=(k == NK - 1),
                    )
                base = t * NH * VW
                nc.vector.tensor_copy(
                    vh[:, base: base + NH * VW].rearrange(
                        "p (h c) -> p h c", c=VW
                    )[:, :, 0:DK],
                    ps[:].rearrange("p (h c) -> p h c", c=DK),
                )

            def qk_proj(p):
                """Project Q and K for pair p (streams x tiles)."""
                for which, xdram in (("q", xq), ("k", xk)):
                    xt = []
                    for k in range(NK):
                        xtile = xin.tile([128, S], bf16, tag="xt")
                        nc.sync.dma_start(
                            xtile[: ksz(k), :], xdram[k * 128: k * 128 + ksz(k), :]
                        )
                        xt.append(xtile)
                    for n in range(NQ):
                        ps = scps.tile([128, 512], fp32, tag="sc")
                        for k in range(NK):
                            nc.tensor.matmul(
                                ps[:],
                                wts[which][k][: ksz(k), p * 128:(p + 1) * 128],
                                xt[k][: ksz(k), n * 512:(n + 1) * 512],
                                start=(k == 0),
                                stop=(k == NK - 1),
                            )
                        if which == "q":
                            nc.vector.tensor_copy(
                                qht[p][:, n * 512:(n + 1) * 512], ps[:]
                            )
                        else:
                            nc.vector.tensor_copy(
                                khtp[p][0:64, n * 512: n * 512 + 512], ps[0:64, :]
                            )
                            nc.vector.tensor_copy(
                                khtp[p][64:128, S + n * 512: S + n * 512 + 512],
                                ps[64:128, :],
                            )

            qk_proj(0)

            # ---- attention, pair-pipelined ----------------------------
            scale = float(1.0 / np.sqrt(DK))
            for p in range(NP):
                for n in range(NQ):
                    qof = n * 512
                    ota = otaps.tile([128, 512], fp32, tag="ota")
                    otb = otbps.tile([128, 512], fp32, tag="otb")
                    for t in range(NT):
                        sa = scps.tile([128, 512], fp32, tag="sc")
                        sb = scps.tile([128, 512], fp32, tag="sc")
                        nc.tensor.matmul(
                            sa[:],
                            khtp[p][:, t * 128:(t + 1) * 128],
                            qht[p][:, qof: qof + 512],
                            start=True, stop=True,
                        )
                        nc.tensor.matmul(
                            sb[:],
                            khtp[p][:, S + t * 128: S + (t + 1) * 128],
                            qht[p][:, qof: qof + 512],
                            start=True, stop=True,
                        )
                        ea = epool.tile([128, 512], bf16, tag="e")
                        eb = epool.tile([128, 512], bf16, tag="e")
                        nc.scalar.activation(
                            ea[:], sa[:], mybir.ActivationFunctionType.Exp,
                            scale=scale,
                        )
                        nc.scalar.activation(
                            eb[:], sb[:], mybir.ActivationFunctionType.Exp,
                            scale=scale,
                        )
                        off = t * 128 - n * 512
                        if 0 <= off < 512:
                            nc.vector.tensor_mul(
                                ea[:, off: off + 128], ea[:, off: off + 128], eye[:]
                            )
                            nc.vector.tensor_mul(
                                eb[:, off: off + 128], eb[:, off: off + 128], eye[:]
                            )
                        vbase = t * NH * VW
                        nc.tensor.matmul(
                            ota[:],
                            vh[:, vbase + (2 * p) * VW: vbase + (2 * p + 1) * VW],
                            ea[:],
                            start=(t == 0), stop=(t == NT - 1),
                        )
                        nc.tensor.matmul(
                            otb[:],
                            vh[:, vbase + (2 * p + 1) * VW: vbase + (2 * p + 2) * VW],
                            eb[:],
                            start=(t == 0), stop=(t == NT - 1),
                        )
                    # normalize; HW partition_broadcast reads physical
                    # partition 0, so bounce the recip rows down via DMA.
                    rd = npool.tile([128, 1024], fp32, tag="rd")
                    nc.vector.reciprocal(rd[64:65, 0:512], ota[64:65, :])
                    nc.vector.reciprocal(rd[64:65, 512:1024], otb[64:65, :])
                    nc.sync.dma_start(rd[0:1, 0:512], rd[64:65, 0:512])
                    nc.sync.dma_start(rd[0:1, 512:1024], rd[64:65, 512:1024])
                    bca = npool.tile([64, 512], fp32, tag="bca")
                    bcb = npool.tile([64, 512], fp32, tag="bcb")
                    nc.gpsimd.partition_broadcast(bca[:], rd[0:1, 0:512], channels=64)
                    nc.gpsimd.partition_broadcast(bcb[:], rd[0:1, 512:1024], channels=64)
                    nc.vector.tensor_mul(
                        ot[p][0:64, qof: qof + 512], ota[0:64, :], bca[:]
                    )
                    tmpb = npool.tile([64, 512], bf16, tag="tmpb")
                    nc.vector.tensor_mul(tmpb[:], otb[0:64, :], bcb[:])
                    nc.sync.dma_start(ot[p][64:128, qof: qof + 512], tmpb[:])
                    if n == 0 and p + 1 < NP:
                        qk_proj(p + 1)  # mid-pair: DMAs prefetch, MMs fill gaps
                    if p == NP - 1 and n > 0:
                        # output projection for the PREVIOUS chunk (its
                        # normalize is long done -> no stall on the chain)
                        for qs in range(4):
                            qt = (n - 1) * 4 + qs
                            osb = opool.tile([128, DOUT], fp32, tag="osb")
                            for nd in range(ND):
                                ps = scps.tile([128, DCH], fp32, tag="sc")
                                for pp in range(NP):
                                    nc.tensor.matmul(
                                        ps[:],
                                        ot[pp][:, qt * 128:(qt + 1) * 128],
                                        wo_sb[:, pp * DOUT + nd * DCH: pp * DOUT + nd * DCH + DCH],
                                        start=(pp == 0), stop=(pp == NP - 1),
                                    )
                                nc.vector.tensor_copy(
                                    osb[:, nd * DCH:(nd + 1) * DCH], ps[:]
                                )
                            nc.sync.dma_start(
                                outp[qt * 128:(qt + 1) * 128, :], osb[:]
                            )

            # ---- output projection for the final q chunk --------------
            for qs in range(4):
                qt = (NQ - 1) * 4 + qs
                osb = opool.tile([128, DOUT], fp32, tag="osb")
                for nd in range(ND):
                    ps = scps.tile([128, DCH], fp32, tag="sc")
                    for pp in range(NP):
                        nc.tensor.matmul(
                            ps[:],
                            ot[pp][:, qt * 128:(qt + 1) * 128],
                            wo_sb[:, pp * DOUT + nd * DCH: pp * DOUT + nd * DCH + DCH],
                            start=(pp == 0), stop=(pp == NP - 1),
                        )
                    nc.vector.tensor_copy(osb[:, nd * DCH:(nd + 1) * DCH], ps[:])
                nc.sync.dma_start(outp[qt * 128:(qt + 1) * 128, :], osb[:])

    nc.compile()
    return nc


def _bf16(a):
    return np.ascontiguousarray(a).astype(ml_dtypes.bfloat16)


def _prep_core_inputs(q, k, v, Wq, bq, Wk, bk, Wv, bv, Wo, aug_bias):
    """Per-core host-side slicing/transposition. Returns list of 8 dicts."""
    eyec = _bf16(1.0 - np.eye(128, dtype=np.float32))
    maps = []
    for c in range(N_CORES):
        b = c // 2
        h0 = (c % 2) * HEADS_PER_CORE
        r0, r1 = h0 * DK, (h0 + HEADS_PER_CORE) * DK
        m = {}
        for name, x in (("xq", q[b]), ("xk", k[b]), ("xv", v[b])):
            xt = x.T  # [D, S]
            if aug_bias:
                xt = np.concatenate([xt, np.ones((1, S), np.float32)], axis=0)
            m[name] = _bf16(xt)
        for name, W, bias in (("wq", Wq, bq), ("wk", Wk, bk), ("wv", Wv, bv)):
            wt = W[r0:r1, :].T  # [D, DC]
            if aug_bias:
                wt = np.concatenate([wt, bias[None, r0:r1]], axis=0)
            m[name] = _bf16(wt)
        m["wo"] = _bf16(Wo[:, r0:r1].T)  # [DC, D]
        m["eyec"] = eyec
        maps.append(m)
    return maps


_PROGRAM_CACHE = {}


def _get_program(aug_bias):
    if aug_bias not in _PROGRAM_CACHE:
        _PROGRAM_CACHE[aug_bias] = build_attention_core(
            S=S, DIN=D, NH=HEADS_PER_CORE, DOUT=D, aug_bias=aug_bias
        )
    return _PROGRAM_CACHE[aug_bias]


def _reference_fallback(q, k, v, Wq, bq, Wk, bk, Wv, bv, Wo, bo, mask):
    """Pure-numpy fallback for unexpected mask patterns."""
    out = np.empty((B, S, D), np.float32)
    msk = np.broadcast_to(mask.reshape(mask.shape[-2], mask.shape[-1]), (S, S))
    for b in range(B):
        qh = (q[b] @ Wq.T + bq).reshape(S, H, DK).transpose(1, 0, 2)
        kh = (k[b] @ Wk.T + bk).reshape(S, H, DK).transpose(1, 0, 2)
        vh = (v[b] @ Wv.T + bv).reshape(S, H, DK).transpose(1, 0, 2)
        acc = np.empty((H, S, DK), np.float32)
        for h in range(H):
            s = (qh[h] @ kh[h].T) / np.float32(np.sqrt(DK))
            s = np.where(msk == 0, np.finfo(np.float32).min, s)
            s = s - s.max(axis=-1, keepdims=True)
            e = np.exp(s)
            p = e / e.sum(axis=-1, keepdims=True)
            acc[h] = p @ vh[h]
        o = acc.transpose(1, 0, 2).reshape(S, D)
        out[b] = o @ Wo.T + bo
    return out


def kernel(q, k, v, Wq, bq, Wk, bk, Wv, bv, Wo, bo, mask, _trace=False):
    from concourse.bass_utils import run_bass_kernel_spmd

    q = np.asarray(q, np.float32)
    k = np.asarray(k, np.float32)
    v = np.asarray(v, np.float32)
    Wq, bq = np.asarray(Wq, np.float32), np.asarray(bq, np.float32)
    Wk, bk = np.asarray(Wk, np.float32), np.asarray(bk, np.float32)
    Wv, bv = np.asarray(Wv, np.float32), np.asarray(bv, np.float32)
    Wo, bo = np.asarray(Wo, np.float32), np.asarray(bo, np.float32)
    mask = np.asarray(mask)

    expected_mask = 1 - np.eye(S, dtype=np.int32)
    if not np.array_equal(mask.reshape(-1, S, S)[0].astype(np.int32), expected_mask):
        return _reference_fallback(q, k, v, Wq, bq, Wk, bk, Wv, bv, Wo, bo, mask)

    aug_bias = bool(np.any(bq) or np.any(bk) or np.any(bv))
    nc = _get_program(aug_bias)
    in_maps = _prep_core_inputs(q, k, v, Wq, bq, Wk, bk, Wv, bv, Wo, aug_bias)
    res = run_bass_kernel_spmd(
        nc, in_maps, core_ids=list(range(N_CORES)), trace=_trace
    )
    out = np.empty((B, S, D), np.float32)
    for b in range(B):
        out[b] = res.results[2 * b]["outp"] + res.results[2 * b + 1]["outp"] + bo
    if _trace:
        kernel.last_results = res
    return out


# revision 21
# speedup vs baseline: 1.0805x; 1.0276x over previous
"""Diagonal-masked multi-head self-attention on 8 TRN2 NeuronCores.

Sharding: core c handles batch b = c // 2 and heads h0 = (c % 2) * 8 .. +8
(data parallel on B=4, tensor parallel over the 16 heads).  Each core
computes a partial output [S, D]; the host sums the two half-head partials
per batch and adds the output bias.

Per-core dataflow (bf16 matmuls, fp32 PSUM accumulation):
  - Host pre-transposes activations/weights so every matmul operand is
    already in its natural [K-on-partitions, free] layout.
  - Q/K projections produce QH^T / KH^T [dk, seq]; V produces VH [seq, dk].
    KH^T is stored twice with the other head's rows zeroed so score
    matmuls run with full K=128 weights (enables fast weight load).
  - Scores are computed transposed (S^T[t, q]); exp runs on the scalar
    engine straight out of PSUM (scale=1/sqrt(dk) folded in); the
    diagonal mask multiplies the one diagonal 128x128 block by (1 - I).
  - P^T @ V is computed as O^T with a ones column folded into the V
    weights, so each head's softmax denominator falls out of the same
    matmul (row 64 of each half's PSUM tile).
  - Normalization broadcasts the reciprocal denominator across
    partitions on GpSimd and multiplies during the PSUM->SBUF copy.
  - The output projection contracts O^T directly (it is already the
    lhsT the PE wants).
"""

import numpy as np
import ml_dtypes

B, S, D, H = 4, 2048, 1024, 16
DK = D // H
N_CORES = 8
HEADS_PER_CORE = H // 2


def build_attention_core(S=2048, DIN=1024, NH=8, DOUT=1024, aug_bias=False):
    """Build the per-core Tile program (strict phases, 1024-wide exp)."""
    import concourse.bacc as bacc
    import concourse.bass as bass
    import concourse.mybir as mybir
    import concourse.tile as tile

    fp32 = mybir.dt.float32
    bf16 = mybir.dt.bfloat16

    NP = NH // 2              # head pairs
    DC = NH * DK              # concat head dim on this core
    VW = 128                  # per-head V slot: [V(64) ones(1) pad(63)=1]
    NT = S // 128             # t tiles (key/value positions)
    NQ = S // 512             # q chunks of 512
    KA = DIN + 1 if aug_bias else DIN
    NK = (KA + 127) // 128    # contraction tiles for projections
    ND = (DOUT + 511) // 512  # output-dim chunks
    DCH = min(512, DOUT)

    assert S % 512 == 0 and DIN % 128 == 0 and DOUT % 512 in (0, DOUT)

    nc = bacc.Bacc(None, target_bir_lowering=False, debug=False)

    xq = nc.dram_tensor("xq", [KA, S], bf16, kind="ExternalInput")
    xk = nc.dram_tensor("xk", [KA, S], bf16, kind="ExternalInput")
    xv = nc.dram_tensor("xv", [KA, S], bf16, kind="ExternalInput")
    wq = nc.dram_tensor("wq", [KA, DC], bf16, kind="ExternalInput")
    wk = nc.dram_tensor("wk", [KA, DC], bf16, kind="ExternalInput")
    wv = nc.dram_tensor("wv", [KA, DC], bf16, kind="ExternalInput")
    wo = nc.dram_tensor("wo", [DC, DOUT], bf16, kind="ExternalInput")
    eyec = nc.dram_tensor("eyec", [128, 128], bf16, kind="ExternalInput")
    outp = nc.dram_tensor("outp", [S, DOUT], fp32, kind="ExternalOutput")

    def ksz(k):  # rows in contraction tile k
        return min(128, KA - k * 128)

    with tile.TileContext(nc) as tc:
        with (
            tc.tile_pool(name="persist", bufs=1) as persist,
            tc.tile_pool(name="xin", bufs=NK) as xin,
            tc.tile_pool(name="win", bufs=1) as win,
            tc.tile_pool(name="epool", bufs=6) as epool,
            tc.tile_pool(name="npool", bufs=2) as npool,
            tc.tile_pool(name="opool", bufs=2) as opool,
        ):
            # ---- persistent SBUF tensors -------------------------------
            qht = persist.tile([128, NP * S], bf16, tag="qht")       # pair-major
            khtp = persist.tile([128, 2 * NP * S], bf16, tag="khtp")  # zero-padded
            vh = persist.tile([128, NT * NH * VW], bf16, tag="vh")
            ot = persist.tile([128, NP * S], bf16, tag="ot")
            eye = persist.tile([128, 128], bf16, tag="eye")
            wo_sb = persist.tile([128, NP * DOUT], bf16, tag="wo")

            nc.sync.dma_start(eye[:], eyec[:])
            nc.vector.memset(vh[:], 1.0)
            nc.vector.memset(khtp[:], 0.0)
            for p in range(NP):
                nc.sync.dma_start(
                    wo_sb[:, p * DOUT:(p + 1) * DOUT],
                    wo[p * 128:(p + 1) * 128, :],
                )

            # ---- phase A: projections ---------------------------------
            with tc.tile_pool(name="projps", bufs=2, space="PSUM") as projps:
                for which, xdram, wdram in (("q", xq, wq), ("k", xk, wk)):
                    xt, wt = [], []
                    for k in range(NK):
                        xtile = xin.tile([128, S], bf16, tag="xt")
                        nc.sync.dma_start(xtile[: ksz(k), :], xdram[k * 128: k * 128 + ksz(k), :])
                        xt.append(xtile)
                        wtile = win.tile([128, DC], bf16, tag=f"w{which}{k}")
                        nc.sync.dma_start(wtile[: ksz(k), :], wdram[k * 128: k * 128 + ksz(k), :])
                        wt.append(wtile)
                    for m in range(NP):
                        for n in range(NQ):
                            ps = projps.tile([128, 512], fp32, tag="proj_ps")
                            for k in range(NK):
                                nc.tensor.matmul(
                                    ps[:],
                                    wt[k][: ksz(k), m * 128:(m + 1) * 128],
                                    xt[k][: ksz(k), n * 512:(n + 1) * 512],
                                    start=(k == 0),
                                    stop=(k == NK - 1),
                                )
                            if which == "q":
                                nc.vector.tensor_copy(
                                    qht[:, m * S + n * 512: m * S + (n + 1) * 512],
                                    ps[:],
                                )
                            else:
                                c0 = (2 * m) * S + n * 512
                                c1 = (2 * m + 1) * S + n * 512
                                nc.vector.tensor_copy(
                                    khtp[0:64, c0: c0 + 512], ps[0:64, :]
                                )
                                nc.vector.tensor_copy(
                                    khtp[64:128, c1: c1 + 512], ps[64:128, :]
                                )

                # V projection
                xt, wt = [], []
                for k in range(NK):
                    xtile = xin.tile([128, S], bf16, tag="xt")
                    nc.sync.dma_start(xtile[: ksz(k), :], xv[k * 128: k * 128 + ksz(k), :])
                    xt.append(xtile)
                    wtile = win.tile([128, DC], bf16, tag=f"wv{k}")
                    nc.sync.dma_start(wtile[: ksz(k), :], wv[k * 128: k * 128 + ksz(k), :])
                    wt.append(wtile)
                for t in range(NT):
                    ps = projps.tile([128, DC], fp32, tag="proj_ps")
                    for k in range(NK):
                        nc.tensor.matmul(
                            ps[:],
                            xt[k][: ksz(k), t * 128:(t + 1) * 128],
                            wt[k][: ksz(k), :],
                            start=(k == 0),
                            stop=(k == NK - 1),
                        )
                    base = t * NH * VW
                    nc.vector.tensor_copy(
                        vh[:, base: base + NH * VW].rearrange(
                            "p (h c) -> p h c", c=VW
                        )[:, :, 0:DK],
                        ps[:].rearrange("p (h c) -> p h c", c=DK),
                    )

            # ---- phase B: attention (one 1024-wide exp per t) ---------
            scale = float(1.0 / np.sqrt(DK))
            with (
                tc.tile_pool(name="scps", bufs=2, space="PSUM") as scps,
                tc.tile_pool(name="otaps", bufs=2, space="PSUM") as otaps,
                tc.tile_pool(name="otbps", bufs=2, space="PSUM") as otbps,
            ):
                for n in range(NQ):
                    for p in range(NP):
                        qof = p * S + n * 512
                        ota = otaps.tile([128, 512], fp32, tag="ota")
                        otb = otbps.tile([128, 512], fp32, tag="otb")
                        for t in range(NT):
                            sc = scps.tile([128, 1024], fp32, tag="sc")
                            nc.tensor.matmul(
                                sc[:, 0:512],
                                khtp[:, (2 * p) * S + t * 128: (2 * p) * S + (t + 1) * 128],
                                qht[:, qof: qof + 512],
                                start=True, stop=True,
                            )
                            nc.tensor.matmul(
                                sc[:, 512:1024],
                                khtp[:, (2 * p + 1) * S + t * 128: (2 * p + 1) * S + (t + 1) * 128],
                                qht[:, qof: qof + 512],
                                start=True, stop=True,
                            )
                            e = epool.tile([128, 1024], bf16, tag="e")
                            nc.scalar.activation(
                                e[:], sc[:], mybir.ActivationFunctionType.Exp,
                                scale=scale,
                            )
                            off = t * 128 - n * 512
                            if 0 <= off < 512:
                                nc.vector.tensor_mul(
                                    e[:, off: off + 128], e[:, off: off + 128], eye[:]
                                )
                                nc.vector.tensor_mul(
                                    e[:, 512 + off: 512 + off + 128],
                                    e[:, 512 + off: 512 + off + 128], eye[:]
                                )
                            vbase = t * NH * VW
                            nc.tensor.matmul(
                                ota[:],
                                vh[:, vbase + (2 * p) * VW: vbase + (2 * p + 1) * VW],
                                e[:, 0:512],
                                start=(t == 0), stop=(t == NT - 1),
                            )
                            nc.tensor.matmul(
                                otb[:],
                                vh[:, vbase + (2 * p + 1) * VW: vbase + (2 * p + 2) * VW],
                                e[:, 512:1024],
                                start=(t == 0), stop=(t == NT - 1),
                            )
                        # normalize (partition_broadcast reads physical p0)
                        rd = npool.tile([128, 1024], fp32, tag="rd")
                        nc.vector.reciprocal(rd[64:65, 0:512], ota[64:65, :])
                        nc.vector.reciprocal(rd[64:65, 512:1024], otb[64:65, :])
                        nc.sync.dma_start(rd[0:1, 0:512], rd[64:65, 0:512])
                        nc.sync.dma_start(rd[0:1, 512:1024], rd[64:65, 512:1024])
                        bca = npool.tile([64, 512], fp32, tag="bca")
                        bcb = npool.tile([64, 512], fp32, tag="bcb")
                        nc.gpsimd.partition_broadcast(bca[:], rd[0:1, 0:512], channels=64)
                        nc.gpsimd.partition_broadcast(bcb[:], rd[0:1, 512:1024], channels=64)
                        nc.vector.tensor_mul(
                            ot[0:64, qof: qof + 512], ota[0:64, :], bca[:]
                        )
                        tmpb = npool.tile([64, 512], bf16, tag="tmpb")
                        nc.vector.tensor_mul(tmpb[:], otb[0:64, :], bcb[:])
                        nc.sync.dma_start(ot[64:128, qof: qof + 512], tmpb[:])

            # ---- phase C: output projection ---------------------------
            with tc.tile_pool(name="outps", bufs=4, space="PSUM") as outps:
                for qt in range(S // 128):
                    osb = opool.tile([128, DOUT], fp32, tag="osb")
                    for nd in range(ND):
                        ps = outps.tile([128, DCH], fp32, tag="out_ps")
                        for p in range(NP):
                            nc.tensor.matmul(
                                ps[:],
                                ot[:, p * S + qt * 128: p * S + (qt + 1) * 128],
                                wo_sb[:, p * DOUT + nd * DCH: p * DOUT + nd * DCH + DCH],
                                start=(p == 0), stop=(p == NP - 1),
                            )
                        nc.vector.tensor_copy(osb[:, nd * DCH:(nd + 1) * DCH], ps[:])
                    nc.sync.dma_start(outp[qt * 128:(qt + 1) * 128, :], osb[:])

    nc.compile()
    return nc


def _bf16(a):
    return np.ascontiguousarray(a).astype(ml_dtypes.bfloat16)


def _prep_core_inputs(q, k, v, Wq, bq, Wk, bk, Wv, bv, Wo, aug_bias):
    """Per-core host-side slicing/transposition. Returns list of 8 dicts."""
    eyec = _bf16(1.0 - np.eye(128, dtype=np.float32))
    maps = []
    for c in range(N_CORES):
        b = c // 2
        h0 = (c % 2) * HEADS_PER_CORE
        r0, r1 = h0 * DK, (h0 + HEADS_PER_CORE) * DK
        m = {}
        for name, x in (("xq", q[b]), ("xk", k[b]), ("xv", v[b])):
            xt = x.T  # [D, S]
            if aug_bias:
                xt = np.concatenate([xt, np.ones((1, S), np.float32)], axis=0)
            m[name] = _bf16(xt)
        for name, W, bias in (("wq", Wq, bq), ("wk", Wk, bk), ("wv", Wv, bv)):
            wt = W[r0:r1, :].T  # [D, DC]
            if aug_bias:
                wt = np.concatenate([wt, bias[None, r0:r1]], axis=0)
            m[name] = _bf16(wt)
        m["wo"] = _bf16(Wo[:, r0:r1].T)  # [DC, D]
        m["eyec"] = eyec
        maps.append(m)
    return maps


_PROGRAM_CACHE = {}


def _get_program(aug_bias):
    if aug_bias not in _PROGRAM_CACHE:
        _PROGRAM_CACHE[aug_bias] = build_attention_core(
            S=S, DIN=D, NH=HEADS_PER_CORE, DOUT=D, aug_bias=aug_bias
        )
    return _PROGRAM_CACHE[aug_bias]


def _reference_fallback(q, k, v, Wq, bq, Wk, bk, Wv, bv, Wo, bo, mask):
    """Pure-numpy fallback for unexpected mask patterns."""
    out = np.empty((B, S, D), np.float32)
    msk = np.broadcast_to(mask.reshape(mask.shape[-2], mask.shape[-1]), (S, S))
    for b in range(B):
        qh = (q[b] @ Wq.T + bq).reshape(S, H, DK).transpose(1, 0, 2)
        kh = (k[b] @ Wk.T + bk).reshape(S, H, DK).transpose(1, 0, 2)
        vh = (v[b] @ Wv.T + bv).reshape(S, H, DK).transpose(1, 0, 2)
        acc = np.empty((H, S, DK), np.float32)
        for h in range(H):
            s = (qh[h] @ kh[h].T) / np.float32(np.sqrt(DK))
            s = np.where(msk == 0, np.finfo(np.float32).min, s)
            s = s - s.max(axis=-1, keepdims=True)
            e = np.exp(s)
            p = e / e.sum(axis=-1, keepdims=True)
            acc[h] = p @ vh[h]
        o = acc.transpose(1, 0, 2).reshape(S, D)
        out[b] = o @ Wo.T + bo
    return out


def kernel(q, k, v, Wq, bq, Wk, bk, Wv, bv, Wo, bo, mask, _trace=False):
    from concourse.bass_utils import run_bass_kernel_spmd

    q = np.asarray(q, np.float32)
    k = np.asarray(k, np.float32)
    v = np.asarray(v, np.float32)
    Wq, bq = np.asarray(Wq, np.float32), np.asarray(bq, np.float32)
    Wk, bk = np.asarray(Wk, np.float32), np.asarray(bk, np.float32)
    Wv, bv = np.asarray(Wv, np.float32), np.asarray(bv, np.float32)
    Wo, bo = np.asarray(Wo, np.float32), np.asarray(bo, np.float32)
    mask = np.asarray(mask)

    expected_mask = 1 - np.eye(S, dtype=np.int32)
    if not np.array_equal(mask.reshape(-1, S, S)[0].astype(np.int32), expected_mask):
        return _reference_fallback(q, k, v, Wq, bq, Wk, bk, Wv, bv, Wo, bo, mask)

    aug_bias = bool(np.any(bq) or np.any(bk) or np.any(bv))
    nc = _get_program(aug_bias)
    in_maps = _prep_core_inputs(q, k, v, Wq, bq, Wk, bk, Wv, bv, Wo, aug_bias)
    res = run_bass_kernel_spmd(
        nc, in_maps, core_ids=list(range(N_CORES)), trace=_trace
    )
    out = np.empty((B, S, D), np.float32)
    for b in range(B):
        out[b] = res.results[2 * b]["outp"] + res.results[2 * b + 1]["outp"] + bo
    if _trace:
        kernel.last_results = res
    return out
